# revision 1
# baseline (speedup 1.0000x reference)
"""ArcFace multi-head-sharded loss on 8 TRN2 NeuronCores.

Strategy: shard the (64, 2048, 256) weight table over the group axis —
each core owns 8 groups (16MB). Samples are routed host-side to the core
owning their group (the host routing replaces the all-to-all). Each core:

  - receives its weight shard pre-transposed to E-major (for TensorE),
  - computes per-class weight norms via square + ones-matmul reduction,
  - computes cos(b, c) = <x_b, w_c> * (1/||x_b||) * (1/||w_c||) with the
    sample-norm applied as a per-partition activation scale and the
    class-norm applied via a broadcast tile,
  - applies the ArcFace margin to the target logit and the CE loss
    per sample entirely on-device (exp with fused accumulation, target
    extraction via iota==label mask),
  - returns a single partial-loss scalar (sum of -logp/B over its samples).

Host: sums the 8 scalars. Total HBM traffic per core ~16MB => memory-bound.

Samples are packed into "bands" of NG=32 partition rows, 128/NG bands per
128-row sample tile; each band is one weight group's samples (padded).
"""

import sys
import numpy as np
import ml_dtypes

BF16 = ml_dtypes.bfloat16

_TRN_REPO = "/opt/trn_rl_repo"
if _TRN_REPO not in sys.path:
    sys.path.insert(0, _TRN_REPO)

# problem config (hardcoded per spec)
B, E, G, C = 512, 256, 64, 2048
NCORES = 8
GPC = G // NCORES        # weight groups per core
NG = 32                  # sample slots per band
BPT = 128 // NG          # bands per 128-partition sample tile
KE = E // 128            # contraction chunks
NCC = C // 512           # 512-col chunks per group
SCALE = 64.0
MARGIN = 0.5
COS_M = float(np.cos(MARGIN))
SIN_M = float(np.sin(MARGIN))
THETA = float(np.cos(np.pi - MARGIN))
SINMM = float(np.sin(np.pi - MARGIN) * MARGIN)
EPS = 1e-12

_graph_cache = {}


def _build(nb):
    """Build the per-core Bass graph for nb weight bands (nb % BPT == 0)."""
    from contextlib import ExitStack
    import concourse.bacc as bacc
    import concourse.tile as tile
    from concourse import mybir

    f32 = mybir.dt.float32
    bf16 = mybir.dt.bfloat16
    i32 = mybir.dt.int32
    A = mybir.AluOpType
    AF = mybir.ActivationFunctionType

    T = nb // BPT
    nc = bacc.Bacc(None)

    wt_ext = nc.declare_dram_parameter("wt", [nb, 128, 2 * C], bf16, isOutput=False)
    x_ext = nc.declare_dram_parameter("x", [T, 128, E], f32, isOutput=False)
    xt_ext = nc.declare_dram_parameter("xt", [T, 128, KE * 128], bf16, isOutput=False)
    lidx_ext = nc.declare_dram_parameter("lidx", [T, 128, 1], f32, isOutput=False)
    vld_ext = nc.declare_dram_parameter("vld", [T, 128, 1], i32, isOutput=False)
    redw_ext = nc.declare_dram_parameter("redw", [T, 128, 1], f32, isOutput=False)
    sel_ext = nc.declare_dram_parameter("sel", [NCC, BPT * NCC, 128], bf16, isOutput=False)
    out_ext = nc.declare_dram_parameter("out", [1, 1], f32, isOutput=True)

    with tile.TileContext(nc) as tc, ExitStack() as ctx:
        wpool = ctx.enter_context(tc.tile_pool(name="w", bufs=5))
        w2pool = ctx.enter_context(tc.tile_pool(name="w2", bufs=3))
        rbpool = ctx.enter_context(tc.tile_pool(name="rb", bufs=2))
        cwpool = ctx.enter_context(tc.tile_pool(name="cw", bufs=2))
        scpool = ctx.enter_context(tc.tile_pool(name="scr", bufs=1))
        cpool = ctx.enter_context(tc.tile_pool(name="const", bufs=1))
        vpool = ctx.enter_context(tc.tile_pool(name="vec", bufs=2))
        pmain = ctx.enter_context(tc.tile_pool(name="pmain", bufs=4, space="PSUM"))
        pnorm = ctx.enter_context(tc.tile_pool(name="pnorm", bufs=2, space="PSUM"))
        pmisc = ctx.enter_context(tc.tile_pool(name="pmisc", bufs=1, space="PSUM"))
        ploss = ctx.enter_context(tc.tile_pool(name="ploss", bufs=1, space="PSUM"))

        # preload the natural_log_exp_and_others ACT table set (exp, ln,
        # square, copy): one resident set => zero mid-kernel table loads
        nc.scalar.add_instruction(mybir.InstLoadActFuncSet(
            name="preload-actset-6", act_func_set_id=6, ins=[], outs=[]))

        # weight stream DMAs are the critical path; interleave the small
        # PE-feeding inputs (xt, sel) right after the first bands' DMAs
        w_tiles = []
        for b in range(nb):
            wt = wpool.tile([128, 2 * C], bf16, tag="wt", name=f"wt{b}")
            w_tiles.append(wt)
        nc.sync.dma_start(out=w_tiles[0][:], in_=wt_ext[0])

        iota_g = cpool.tile([128, C], f32, tag="iotag")
        ones_bc = cpool.tile([128, NG], bf16, tag="onesbc")
        nc.vector.memset(ones_bc[:], 1.0)
        sel_sb = cpool.tile([BPT * NCC, NCC * 128], bf16, tag="sel")
        xt_sb = []
        for t in range(T):
            xts = cpool.tile([128, KE * 128], bf16, tag=f"xt{t}", name=f"xts{t}")
            nc.sync.dma_start(out=xts[:], in_=xt_ext[t])
            xt_sb.append(xts)
        for cc in range(NCC):
            nc.sync.dma_start(out=sel_sb[:, 128 * cc:128 * (cc + 1)], in_=sel_ext[cc])
        for b in range(1, nb):
            nc.sync.dma_start(out=w_tiles[b][:], in_=wt_ext[b])

        # small per-tile inputs + x-norm pipeline (1/||x|| = exp(-0.5*ln(n2));
        # pad rows are ones so n2 > 0)
        x_sb, lidx_sb, vld_sb, redw_sb = [], [], [], []
        rinv_x, sc64 = [], []
        for t in range(T):
            xs = cpool.tile([128, E], f32, tag=f"x{t}")
            nc.sync.dma_start(out=xs[:], in_=x_ext[t])
            x_sb.append(xs)
            ls = cpool.tile([128, 1], f32, tag=f"li{t}")
            nc.sync.dma_start(out=ls[:], in_=lidx_ext[t])
            lidx_sb.append(ls)
            vs = cpool.tile([128, 1], i32, tag=f"vl{t}")
            nc.sync.dma_start(out=vs[:], in_=vld_ext[t])
            vld_sb.append(vs)
            rs = cpool.tile([128, 1], f32, tag=f"rw{t}")
            nc.sync.dma_start(out=rs[:], in_=redw_ext[t])
            redw_sb.append(rs)

            xsq = vpool.tile([128, E], f32, tag="xsq")
            xn2 = cpool.tile([128, 1], f32, tag=f"xn2{t}")
            nc.vector.tensor_tensor(xsq[:], xs[:], xs[:], A.mult)
            nc.vector.reduce_sum(xn2[:], xsq[:], axis=mybir.AxisListType.X)
            xln = cpool.tile([128, 1], f32, tag=f"xln{t}")
            nc.scalar.activation(xln[:], xn2[:], AF.Ln)
            rx = cpool.tile([128, 1], f32, tag=f"rx{t}")
            nc.scalar.activation(rx[:], xln[:], AF.Exp, scale=-0.5)
            rinv_x.append(rx)
            s64 = cpool.tile([128, 1], f32, tag=f"s64{t}")
            nc.vector.tensor_scalar_mul(s64[:], rx[:], SCALE)
            sc64.append(s64)

        # iota (GpSimd) after its DMA issues; DVE copy so consumers wait on
        # one engine
        nc.gpsimd.iota(iota_g[:], pattern=[[1, C]], base=0, channel_multiplier=0,
                       allow_small_or_imprecise_dtypes=True)
        iota_t = cpool.tile([128, C], f32, tag="iota")
        nc.vector.tensor_copy(iota_t[:], iota_g[:])

        # masks are weight-independent: build during the weight stream
        # (Bacc's generate_event_semaphores legalizes TensorScalarPtr waits)
        masks = []
        for t in range(T):
            maskf = scpool.tile([128, C], bf16, tag="maskf", name=f"maskf{t}", bufs=T)
            nc.vector.tensor_scalar(maskf[:], iota_t[:], lidx_sb[t][:], None, op0=A.is_equal)
            masks.append(maskf)

        # weight stream: per band, main matmuls first (PE-ready as soon as
        # the band's weights land), then squares + norm rows; the per-tile
        # epilogue is emitted inline at each tile boundary so it overlaps the
        # next tile's stream.
        nct = [cpool.tile([BPT * NCC, 512], f32, tag=f"nct{t}", name=f"nct{t}")
               for t in range(T)]
        nsb_t = [w2pool.tile([128, BPT * 512], f32, tag="nsb", bufs=T, name=f"nsb{t}")
                 for t in range(T)]
        loss_ps = ploss.tile([1, 1], f32, tag="loss")
        cps_t = {}

        def emit_tile_mains(t, cc_outer=False):
            """deferred main matmuls for tiles beyond the first; cc_outer
            completes one PSUM chunk at a time so the multiply pipeline can
            start before the whole tile is done"""
            cps_t[t] = [pmain.tile([128, 512], f32, tag="cos", name=f"cos{t}_{cc}")
                        for cc in range(NCC)]
            cps = cps_t[t]
            order = ([(cc, j, k) for cc in range(NCC) for j in range(BPT) for k in range(KE)]
                     if cc_outer else
                     [(cc, j, k) for j in range(BPT) for k in range(KE) for cc in range(NCC)])
            for cc, j, k in order:
                nc.tensor.matmul(
                    cps[cc][NG * j:NG * (j + 1), :],
                    xt_sb[t][:, k * 128 + NG * j: k * 128 + NG * (j + 1)],
                    w_tiles[BPT * t + j][:, k * C + 512 * cc: k * C + 512 * cc + 512],
                    start=(k == 0), stop=(k == KE - 1),
                    tile_position=(0, NG * j),
                )

        def emit_band_norms(b):
            """squares + norm rows only (mains deferred until PSUM frees)"""
            t, j = b // BPT, b % BPT
            wt = w_tiles[b]
            w2 = w2pool.tile([128, 2 * C], bf16, tag="w2", name=f"w2_{b}")
            nc.scalar.activation(w2[:, 0:1536], wt[:, 0:1536], AF.Square)
            nc.vector.tensor_tensor(w2[:, 1536:2816], wt[:, 1536:2816], wt[:, 1536:2816], A.mult)
            nc.gpsimd.tensor_tensor(w2[:, 2816:4096], wt[:, 2816:4096], wt[:, 2816:4096], A.mult)
            nrow = pnorm.tile([128, 512], f32, tag="nrow", name=f"nrow{b}")
            for cc in range(NCC):
                for k in range(KE):
                    nc.tensor.matmul(
                        nrow[NG * cc:NG * (cc + 1), :],
                        ones_bc[:],
                        w2[:, k * C + 512 * cc: k * C + 512 * cc + 512],
                        start=(k == 0), stop=(k == KE - 1),
                        tile_position=(0, NG * cc),
                    )
            nc.vector.tensor_copy(nsb_t[t][:, 512 * j:512 * (j + 1)], nrow[:])

        def emit_tile_norm_rb(t):
            # compact norms -> 1/||w|| (Ln/Exp keeps one ACT table set) -> rb
            nc.sync.dma_start(out=nct[t][:], in_=nsb_t[t][0:128:NG, :])
            nc.scalar.activation(nct[t][:], nct[t][:], AF.Ln)
            rinvb = cpool.tile([BPT * NCC, 512], bf16, tag=f"rinvb{t}", name=f"rinvb{t}")
            nc.scalar.activation(rinvb[:], nct[t][:], AF.Exp, scale=-0.5)
            rb = rbpool.tile([128, C], f32, tag="rb", name=f"rb{t}")
            for cc in range(NCC):
                sl = slice(512 * cc, 512 * (cc + 1))
                rbps = pmisc.tile([128, 512], f32, tag="rbps", name=f"rbps{t}_{cc}")
                nc.tensor.matmul(
                    rbps[:], sel_sb[:, 128 * cc:128 * (cc + 1)], rinvb[:],
                    start=True, stop=True,
                )
                nc.vector.tensor_copy(rb[:, sl], rbps[:])
            return rb

        def emit_tile_epilogue(t, rb):
            cps = cps_t[t]
            coswn = cwpool.tile([128, C], bf16, tag="coswn", name=f"coswn{t}")
            expscr = scpool.tile([128, C], bf16, tag="expscr", name=f"expscr{t}")
            maskf = masks[t]
            for cc in range(NCC):
                sl = slice(512 * cc, 512 * (cc + 1))
                nc.vector.tensor_tensor(coswn[:, sl], cps[cc][:], rb[:, sl], A.mult)
            sumexp = cpool.tile([128, 1], f32, tag=f"se{t}", name=f"se{t}")
            nc.scalar.activation(
                expscr[:], coswn[:], AF.Exp, scale=sc64[t][:], accum_out=sumexp[:],
            )
            traw = cpool.tile([128, 1], f32, tag=f"traw{t}", name=f"traw{t}")
            nc.vector.tensor_tensor(expscr[:], coswn[:], maskf[:], A.mult)
            nc.vector.reduce_sum(traw[:], expscr[:], axis=mybir.AxisListType.X)
            tcos = vpool.tile([128, 1], f32, tag="tcos")
            nc.vector.tensor_tensor(tcos[:], traw[:], rinv_x[t][:], A.mult)
            # margin: ft = t>theta ? t*cos_m - sqrt(1-t^2)*sin_m : t - sinmm
            t2 = vpool.tile([128, 1], f32, tag="t2")
            nc.vector.tensor_tensor(t2[:], tcos[:], tcos[:], A.mult)
            nc.vector.tensor_scalar(t2[:], t2[:], -1.0, 1.0, op0=A.mult, op1=A.add)
            nc.vector.tensor_scalar_max(t2[:], t2[:], 0.0)
            # sin_t = z*rsqrt(z): Quake seed + 2 Newton iterations on DVE
            yrs = vpool.tile([128, 1], f32, tag="yrs")
            yi = yrs.bitcast(i32)
            nc.vector.tensor_scalar(yi[:], t2.bitcast(i32)[:], 1, None, op0=A.arith_shift_right)
            nc.vector.tensor_scalar(yi[:], yi[:], -1, 0x5F3759DF, op0=A.mult, op1=A.add)
            hz = vpool.tile([128, 1], f32, tag="hz")
            nc.vector.tensor_scalar_mul(hz[:], t2[:], 0.5)
            y2 = vpool.tile([128, 1], f32, tag="y2")
            for _ in range(2):
                nc.vector.tensor_tensor(y2[:], yrs[:], yrs[:], A.mult)
                nc.vector.tensor_tensor(y2[:], y2[:], hz[:], A.mult)
                nc.vector.tensor_scalar(y2[:], y2[:], -1.0, 1.5, op0=A.mult, op1=A.add)
                nc.vector.tensor_tensor(yrs[:], yrs[:], y2[:], A.mult)
            sint = vpool.tile([128, 1], f32, tag="sint")
            nc.vector.tensor_tensor(sint[:], t2[:], yrs[:], A.mult)
            ctm = vpool.tile([128, 1], f32, tag="ctm")
            nc.vector.tensor_scalar_mul(ctm[:], tcos[:], COS_M)
            sinm = vpool.tile([128, 1], f32, tag="sinm")
            nc.vector.tensor_scalar_mul(sinm[:], sint[:], SIN_M)
            nc.vector.tensor_tensor(ctm[:], ctm[:], sinm[:], A.subtract)
            tms = vpool.tile([128, 1], f32, tag="tms")
            nc.vector.tensor_scalar_add(tms[:], tcos[:], -SINMM)
            gt = vpool.tile([128, 1], i32, tag="gt")
            nc.vector.tensor_scalar(gt[:], tcos[:], THETA, None, op0=A.is_gt)
            ft = vpool.tile([128, 1], f32, tag="ft")
            nc.vector.select(ft[:], gt[:], ctm[:], tms[:])
            ftv = vpool.tile([128, 1], f32, tag="ftv")
            nc.vector.select(ftv[:], vld_sb[t][:], ft[:], tcos[:])
            tf64 = vpool.tile([128, 2], f32, tag="tf64")
            nc.vector.tensor_scalar_mul(tf64[:, 0:1], tcos[:], SCALE)
            nc.vector.tensor_scalar_mul(tf64[:, 1:2], ftv[:], SCALE)
            ft64 = tf64[:, 1:2]
            eb = vpool.tile([128, 2], f32, tag="eb")
            nc.scalar.activation(eb[:], tf64[:], AF.Exp)
            se2 = vpool.tile([128, 1], f32, tag="se2")
            nc.vector.tensor_tensor(se2[:], sumexp[:], eb[:, 0:1], A.subtract)
            nc.vector.tensor_tensor(se2[:], se2[:], eb[:, 1:2], A.add)
            lse = vpool.tile([128, 1], f32, tag="lse")
            nc.scalar.activation(lse[:], se2[:], AF.Ln)
            lb = cpool.tile([128, 1], f32, tag=f"lb{t}", name=f"lb{t}")
            nc.vector.tensor_tensor(lb[:], lse[:], ft64[:], A.subtract)
            nc.tensor.matmul(
                loss_ps[:], redw_sb[t][:], lb[:],
                start=(t == 0), stop=(t == T - 1),
            )

        # emission order: tile0 mains dense (keeps the PE clock warm), then
        # tile0+tile1 squares/norms, tile0 epilogue, tile1 mains (PSUM-gated),
        # tile1 epilogue
        emit_tile_mains(0)
        for b in range(BPT):
            emit_band_norms(b)
        rb0 = emit_tile_norm_rb(0)
        emit_tile_epilogue(0, rb0)
        for t in range(1, T):
            for j in range(BPT):
                emit_band_norms(BPT * t + j)
            rbt = emit_tile_norm_rb(t)
            emit_tile_mains(t, cc_outer=True)
            emit_tile_epilogue(t, rbt)

        loss_sb = cpool.tile([1, 1], f32, tag="losssb")
        nc.vector.tensor_copy(loss_sb[:], loss_ps[:])
        nc.sync.dma_start(out=out_ext[:], in_=loss_sb[:])

    nc.compile()
    return nc


def _pack(logits, labels, weight):
    """Route samples to the core owning their group; build per-core inputs."""
    logits = np.asarray(logits, dtype=np.float32)
    labels = np.asarray(labels).astype(np.int64)
    weight = np.asarray(weight, dtype=np.float32)

    group = labels // C
    local = (labels % C).astype(np.int32)
    core = group // GPC
    gl = group % GPC

    # band assignment: per (core, local-group), ceil(count/NG) bands
    percg = [[np.nonzero((core == c) & (gl == g))[0] for g in range(GPC)]
             for c in range(NCORES)]
    nbands = [sum(max(1, -(-len(idx) // NG)) for idx in percg[c])
              for c in range(NCORES)]
    nb = max(nbands)
    nb = -(-nb // BPT) * BPT  # round up to full sample tiles
    T = nb // BPT

    in_maps = []
    for c in range(NCORES):
        # band -> (group, sample indices)
        bands = []
        for g in range(GPC):
            idx = percg[c][g]
            nslice = max(1, -(-len(idx) // NG))
            for s in range(nslice):
                bands.append((g, idx[s * NG:(s + 1) * NG]))
        while len(bands) < nb:
            bands.append((0, np.empty(0, dtype=np.int64)))

        wt = np.empty((nb, 128, 2 * C), dtype=BF16)
        x = np.ones((T, 128, E), dtype=np.float32)
        lidx = np.zeros((T, 128, 1), dtype=np.float32)
        vld = np.ones((T, 128, 1), dtype=np.int32)
        redw = np.zeros((T, 128, 1), dtype=np.float32)
        for b, (g, idx) in enumerate(bands):
            wg = weight[c * GPC + g]                     # (C, E)
            for k in range(KE):
                wt[b, :, k * C:(k + 1) * C] = wg[:, k * 128:(k + 1) * 128].T
            t, j = b // BPT, b % BPT
            sl = slice(NG * j, NG * j + len(idx))
            x[t, sl, :] = logits[idx]
            lidx[t, sl, 0] = local[idx]
            vld[t, sl, 0] = (labels[idx] != -1).astype(np.int32)
            redw[t, sl, 0] = 1.0 / B
        sel = np.zeros((NCC, BPT * NCC, 128), dtype=BF16)
        for cc in range(NCC):
            for m in range(128):
                sel[cc, NCC * cc + (m // NG), m] = 1.0
        xt = np.ascontiguousarray(
            np.transpose(x.reshape(T, 128, KE, 128), (0, 3, 2, 1))
            .reshape(T, 128, KE * 128)).astype(BF16)
        in_maps.append({
            "wt": wt, "x": x, "xt": xt,
            "lidx": lidx, "vld": vld, "redw": redw, "sel": sel,
        })
    return in_maps, nb


def _run(logits, labels, weight, trace=False, **kw):
    from concourse.bass_utils import run_bass_kernel_spmd

    in_maps, nb = _pack(logits, labels, weight)
    nc = _graph_cache.get(nb)
    if nc is None:
        nc = _build(nb)
        _graph_cache[nb] = nc
    res = run_bass_kernel_spmd(nc, in_maps, core_ids=list(range(NCORES)),
                               trace=trace, **kw)
    total = sum(float(res.results[i]["out"][0, 0]) for i in range(NCORES))
    return np.asarray(total, dtype=np.float32), res


def kernel(logits, labels, weight):
    loss, _ = _run(logits, labels, weight)
    return loss



# revision 2
# speedup vs baseline: 2.0792x; 2.0792x over previous
"""ArcFace multi-head-sharded loss on 8 TRN2 NeuronCores.

Strategy: shard the (64, 2048, 256) weight table over the group axis —
each core owns 8 groups. Samples are routed host-side to the core owning
their group (host routing replaces the all-to-all). Weight rows are
l2-normalized host-side and quantized to fp8e4 (x16 pre-scale to stay in
the normal range), so the device only does:

  - stream its 8 weight groups (4MB fp8) from HBM,
  - mains: cos_raw(b, c) = <xq_b, wq_c> on PE (fp8 x fp8 -> f32 PSUM),
  - exp with fused accumulation over the class axis (scale folds the
    1/256 quantization scale and the ArcFace scale 64),
  - target logit via a per-row dot with the host-gathered target weight
    row (xw . wtar, 256-wide DVE reduce),
  - the margin + CE epilogue on [128,1] vectors,
  - one partial-loss scalar out (sum of -logp/B over its samples).

Host: sums the 8 scalars. ~4MB HBM traffic per core => memory-bound.

Samples are packed into "bands" of NG=32 partition rows, one band per
weight group (plus overflow bands), 4 bands per 128-row sample tile.
"""

import sys
import numpy as np
import ml_dtypes

BF16 = ml_dtypes.bfloat16
FP8 = ml_dtypes.float8_e4m3

_TRN_REPO = "/opt/trn_rl_repo"
if _TRN_REPO not in sys.path:
    sys.path.insert(0, _TRN_REPO)

# problem config (hardcoded per spec)
B, E, G, C = 512, 256, 64, 2048
NCORES = 8
GPC = G // NCORES        # weight groups per core
NG = 32                  # sample slots per band
BPT = 128 // NG          # bands per 128-partition sample tile
KE = E // 128            # contraction chunks
NCC = C // 512           # 512-col chunks per group
SCALE = 64.0
MARGIN = 0.5
COS_M = float(np.cos(MARGIN))
SIN_M = float(np.sin(MARGIN))
THETA = float(np.cos(np.pi - MARGIN))
SINMM = float(np.sin(np.pi - MARGIN) * MARGIN)
EPS = 1e-12
WS = 16.0                # fp8 pre-scale (per operand); PSUM = WS^2 * cos
DOUBLE_ROW = False       # fp8 DoubleRow perf mode for the mains

_graph_cache = {}


def _build(nb, double_row=DOUBLE_ROW):
    """Build the per-core Bass graph for nb weight bands (nb % BPT == 0)."""
    from contextlib import ExitStack
    import concourse.bacc as bacc
    import concourse.tile as tile
    from concourse import mybir

    f32 = mybir.dt.float32
    bf16 = mybir.dt.bfloat16
    fp8 = mybir.dt.float8e4
    i32 = mybir.dt.int32
    A = mybir.AluOpType
    AF = mybir.ActivationFunctionType

    T = nb // BPT
    nc = bacc.Bacc(None)

    wt_ext = nc.declare_dram_parameter("wt", [nb, 128, KE, C], fp8, isOutput=False)
    xt_ext = nc.declare_dram_parameter("xt", [T, 128, KE, 128], fp8, isOutput=False)
    xw_ext = nc.declare_dram_parameter("xw", [T, 128, E], bf16, isOutput=False)
    wtar_ext = nc.declare_dram_parameter("wtar", [T, 128, E], bf16, isOutput=False)
    redw_ext = nc.declare_dram_parameter("redw", [T, 128, 1], f32, isOutput=False)
    out_ext = nc.declare_dram_parameter("out", [1, 1], f32, isOutput=True)

    with tile.TileContext(nc) as tc, ExitStack() as ctx:
        wpool = ctx.enter_context(tc.tile_pool(name="w", bufs=nb))
        cpool = ctx.enter_context(tc.tile_pool(name="const", bufs=1))
        vpool = ctx.enter_context(tc.tile_pool(name="vec", bufs=2))
        pmain = ctx.enter_context(tc.tile_pool(name="pmain", bufs=4, space="PSUM"))
        ploss = ctx.enter_context(tc.tile_pool(name="ploss", bufs=1, space="PSUM"))

        # preload the natural_log_exp_and_others ACT table set (exp, ln):
        # one resident set => zero mid-kernel table loads
        nc.scalar.add_instruction(mybir.InstLoadActFuncSet(
            name="preload-actset-6", act_func_set_id=6, ins=[], outs=[]))

        # weight stream DMAs are the critical path; interleave the small
        # PE/DVE-feeding inputs right after the first band's DMA
        w_tiles = []
        for b in range(nb):
            wt = wpool.tile([128, KE, C], fp8, tag="wt", name=f"wt{b}")
            w_tiles.append(wt)
        nc.sync.dma_start(out=w_tiles[0][:], in_=wt_ext[0])

        xt_sb, xw_sb, wtar_sb, redw_sb = [], [], [], []
        for t in range(T):
            xts = cpool.tile([128, KE, 128], fp8, tag=f"xt{t}")
            nc.sync.dma_start(out=xts[:], in_=xt_ext[t])
            xt_sb.append(xts)
            xws = cpool.tile([128, E], bf16, tag=f"xw{t}")
            nc.sync.dma_start(out=xws[:], in_=xw_ext[t])
            xw_sb.append(xws)
            wts = cpool.tile([128, E], bf16, tag=f"wtar{t}")
            nc.sync.dma_start(out=wts[:], in_=wtar_ext[t])
            wtar_sb.append(wts)
            rs = cpool.tile([128, 1], f32, tag=f"rw{t}")
            nc.sync.dma_start(out=rs[:], in_=redw_ext[t])
            redw_sb.append(rs)
        for b in range(1, nb):
            nc.sync.dma_start(out=w_tiles[b][:], in_=wt_ext[b])

        # per-tile margin pre-compute: depends only on xw/wtar (DMA'd
        # inputs), so it runs while the weight stream + mains are going.
        # t = <xn, wn_target>; ft = t>theta ? t*cos_m - sqrt(1-t^2)*sin_m
        #                                   : t - sinmm   (always valid)
        eb_t, ft64_t = [], []
        for t in range(T):
            tscr = vpool.tile([128, E], f32, tag="tscr")
            nc.vector.tensor_tensor(tscr[:], xw_sb[t][:], wtar_sb[t][:], A.mult)
            tcos = cpool.tile([128, 1], f32, tag=f"tcos{t}")
            nc.vector.reduce_sum(tcos[:], tscr[:], axis=mybir.AxisListType.X)
            t2 = vpool.tile([128, 1], f32, tag="t2")
            nc.vector.tensor_tensor(t2[:], tcos[:], tcos[:], A.mult)
            nc.vector.tensor_scalar(t2[:], t2[:], -1.0, 1.0, op0=A.mult, op1=A.add)
            nc.vector.tensor_scalar_max(t2[:], t2[:], 0.0)
            # sin_t = z*rsqrt(z): Quake seed + 2 Newton iterations on DVE
            yrs = vpool.tile([128, 1], f32, tag="yrs")
            yi = yrs.bitcast(i32)
            nc.vector.tensor_scalar(yi[:], t2.bitcast(i32)[:], 1, None, op0=A.arith_shift_right)
            nc.vector.tensor_scalar(yi[:], yi[:], -1, 0x5F3759DF, op0=A.mult, op1=A.add)
            hz = vpool.tile([128, 1], f32, tag="hz")
            nc.vector.tensor_scalar_mul(hz[:], t2[:], 0.5)
            y2 = vpool.tile([128, 1], f32, tag="y2")
            for _ in range(2):
                nc.vector.tensor_tensor(y2[:], yrs[:], yrs[:], A.mult)
                nc.vector.tensor_tensor(y2[:], y2[:], hz[:], A.mult)
                nc.vector.tensor_scalar(y2[:], y2[:], -1.0, 1.5, op0=A.mult, op1=A.add)
                nc.vector.tensor_tensor(yrs[:], yrs[:], y2[:], A.mult)
            sint = vpool.tile([128, 1], f32, tag="sint")
            nc.vector.tensor_tensor(sint[:], t2[:], yrs[:], A.mult)
            ctm = vpool.tile([128, 1], f32, tag="ctm")
            nc.vector.tensor_scalar_mul(ctm[:], tcos[:], COS_M)
            sinm = vpool.tile([128, 1], f32, tag="sinm")
            nc.vector.tensor_scalar_mul(sinm[:], sint[:], SIN_M)
            nc.vector.tensor_tensor(ctm[:], ctm[:], sinm[:], A.subtract)
            tms = vpool.tile([128, 1], f32, tag="tms")
            nc.vector.tensor_scalar_add(tms[:], tcos[:], -SINMM)
            gt = vpool.tile([128, 1], i32, tag="gt")
            nc.vector.tensor_scalar(gt[:], tcos[:], THETA, None, op0=A.is_gt)
            ft = vpool.tile([128, 1], f32, tag="ft")
            nc.vector.select(ft[:], gt[:], ctm[:], tms[:])
            tf64 = vpool.tile([128, 2], f32, tag="tf64")
            nc.vector.tensor_scalar_mul(tf64[:, 0:1], tcos[:], SCALE)
            nc.vector.tensor_scalar_mul(tf64[:, 1:2], ft[:], SCALE)
            ft64 = cpool.tile([128, 1], f32, tag=f"ft64{t}")
            nc.vector.tensor_copy(ft64[:], tf64[:, 1:2])
            eb = cpool.tile([128, 2], f32, tag=f"eb{t}")
            nc.scalar.activation(eb[:], tf64[:], AF.Exp)
            eb_t.append(eb)
            ft64_t.append(ft64)

        loss_ps = ploss.tile([1, 1], f32, tag="loss")
        escale = SCALE / (WS * WS)   # exp(escale * psum) = exp(64*cos)

        def emit_tile(t, cc_outer):
            """mains + exp/accum + CE epilogue for sample tile t"""
            cps = [pmain.tile([128, 512], f32, tag="cos", name=f"cos{t}_{cc}")
                   for cc in range(NCC)]
            if double_row:
                order = ([(cc, j) for cc in range(NCC) for j in range(BPT)]
                         if cc_outer else
                         [(cc, j) for j in range(BPT) for cc in range(NCC)])
                for cc, j in order:
                    nc.tensor.matmul(
                        cps[cc][NG * j:NG * (j + 1), :],
                        xt_sb[t][:, 0:KE, NG * j: NG * (j + 1)],
                        w_tiles[BPT * t + j][:, 0:KE, 512 * cc: 512 * cc + 512],
                        start=True, stop=True,
                        perf_mode=mybir.MatmulPerfMode.DoubleRow,
                        tile_position=(0, NG * j),
                    )
            else:
                order = ([(cc, j, k) for cc in range(NCC) for j in range(BPT) for k in range(KE)]
                         if cc_outer else
                         [(cc, j, k) for j in range(BPT) for k in range(KE) for cc in range(NCC)])
                for cc, j, k in order:
                    nc.tensor.matmul(
                        cps[cc][NG * j:NG * (j + 1), :],
                        xt_sb[t][:, k, NG * j: NG * (j + 1)],
                        w_tiles[BPT * t + j][:, k, 512 * cc: 512 * cc + 512],
                        start=(k == 0), stop=(k == KE - 1),
                        tile_position=(0, NG * j),
                    )
            # exp with fused class-axis accumulation, one per PSUM chunk
            ses = cpool.tile([128, NCC], f32, tag=f"ses{t}")
            for cc in range(NCC):
                escr = vpool.tile([128, 512], bf16, tag="escr")
                nc.scalar.activation(escr[:], cps[cc][:], AF.Exp, scale=escale,
                                     accum_out=ses[:, cc:cc + 1])
            sumexp = vpool.tile([128, 1], f32, tag="sumexp")
            nc.vector.reduce_sum(sumexp[:], ses[:], axis=mybir.AxisListType.X)
            # se2 = sumexp - exp(64 t) + exp(64 ft);  lb = ln(se2) - 64 ft
            se2 = vpool.tile([128, 1], f32, tag="se2")
            nc.vector.tensor_tensor(se2[:], sumexp[:], eb_t[t][:, 0:1], A.subtract)
            nc.vector.tensor_tensor(se2[:], se2[:], eb_t[t][:, 1:2], A.add)
            lse = vpool.tile([128, 1], f32, tag="lse")
            nc.scalar.activation(lse[:], se2[:], AF.Ln)
            lb = cpool.tile([128, 1], f32, tag=f"lb{t}")
            nc.vector.tensor_tensor(lb[:], lse[:], ft64_t[t][:], A.subtract)
            nc.tensor.matmul(
                loss_ps[:], redw_sb[t][:], lb[:],
                start=(t == 0), stop=(t == T - 1),
            )

        emit_tile(0, cc_outer=False)
        for t in range(1, T):
            emit_tile(t, cc_outer=True)

        loss_sb = cpool.tile([1, 1], f32, tag="losssb")
        nc.vector.tensor_copy(loss_sb[:], loss_ps[:])
        nc.sync.dma_start(out=out_ext[:], in_=loss_sb[:])

    nc.compile()
    return nc


def _pack(logits, labels, weight):
    """Route samples to the core owning their group; build per-core inputs."""
    logits = np.asarray(logits, dtype=np.float32)
    labels = np.asarray(labels).astype(np.int64)
    weight = np.asarray(weight, dtype=np.float32)

    group = (labels // C).astype(np.int64)
    local = (labels % C).astype(np.int64)
    core = group // GPC
    gl = group % GPC

    # host-side l2 normalization + fp8 quantization (x16 keeps the values
    # in fp8e4's normal range; cos is invariant to the row scaling)
    xn = logits / np.maximum(
        np.sqrt(np.sum(logits * logits, axis=1, keepdims=True)), EPS)
    wn2 = np.sqrt(np.einsum("gce,gce->gc", weight, weight))[:, :, None]
    wn = weight / np.maximum(wn2, EPS)
    wq = (WS * wn).astype(FP8)                    # (G, C, E) fp8 table
    xq = (WS * xn).astype(FP8)                    # (B, E)
    xw_all = (xq.astype(np.float32) / WS).astype(BF16)
    wtar_all = (wq[group, local].astype(np.float32) / WS).astype(BF16)

    # band assignment: per (core, local-group), ceil(count/NG) bands
    percg = [[np.nonzero((core == c) & (gl == g))[0] for g in range(GPC)]
             for c in range(NCORES)]
    nbands = [sum(max(1, -(-len(idx) // NG)) for idx in percg[c])
              for c in range(NCORES)]
    nb = max(nbands)
    nb = -(-nb // BPT) * BPT  # round up to full sample tiles
    T = nb // BPT

    in_maps = []
    for c in range(NCORES):
        # band -> (group, sample indices)
        bands = []
        for g in range(GPC):
            idx = percg[c][g]
            nslice = max(1, -(-len(idx) // NG))
            for s in range(nslice):
                bands.append((g, idx[s * NG:(s + 1) * NG]))
        while len(bands) < nb:
            bands.append((0, np.empty(0, dtype=np.int64)))

        wt = np.empty((nb, 128, KE, C), dtype=FP8)
        xqp = np.zeros((T, 128, E), dtype=FP8)
        xw = np.zeros((T, 128, E), dtype=BF16)
        wtar = np.zeros((T, 128, E), dtype=BF16)
        redw = np.zeros((T, 128, 1), dtype=np.float32)
        for b, (g, idx) in enumerate(bands):
            wg = wq[c * GPC + g]                     # (C, E) fp8
            for k in range(KE):
                wt[b, :, k, :] = wg[:, k * 128:(k + 1) * 128].T
            t, j = b // BPT, b % BPT
            sl = slice(NG * j, NG * j + len(idx))
            xqp[t, sl, :] = xq[idx]
            xw[t, sl, :] = xw_all[idx]
            wtar[t, sl, :] = wtar_all[idx]
            redw[t, sl, 0] = 1.0 / B
        xt = np.ascontiguousarray(
            np.transpose(xqp.reshape(T, 128, KE, 128), (0, 3, 2, 1)))
        in_maps.append({
            "wt": wt, "xt": xt, "xw": xw, "wtar": wtar, "redw": redw,
        })
    return in_maps, nb


def _run(logits, labels, weight, trace=False, **kw):
    from concourse.bass_utils import run_bass_kernel_spmd

    in_maps, nb = _pack(logits, labels, weight)
    nc = _graph_cache.get(nb)
    if nc is None:
        nc = _build(nb)
        _graph_cache[nb] = nc
    res = run_bass_kernel_spmd(nc, in_maps, core_ids=list(range(NCORES)),
                               trace=trace, **kw)
    total = sum(float(res.results[i]["out"][0, 0]) for i in range(NCORES))
    return np.asarray(total, dtype=np.float32), res


def kernel(logits, labels, weight):
    loss, _ = _run(logits, labels, weight)
    return loss


# revision 3
# speedup vs baseline: 2.7027x; 1.2998x over previous
"""ArcFace multi-head-sharded loss on 8 TRN2 NeuronCores.

Strategy: shard the (64, 2048, 256) weight table over the group axis —
each core owns 8 groups. Samples are routed host-side to the core owning
their group (host routing replaces the all-to-all). Weight rows are
l2-normalized host-side and quantized to fp8e4 (x16 pre-scale to stay in
the normal range), so the device only does:

  - stream its 8 weight groups (4MB fp8) from HBM; DMA triggers alternate
    between the two HW-DGE queues (sync + scalar) so descriptor
    generation is not serialized on one engine,
  - mains: cos_raw(b, c) = <xq_b, wq_c> on PE (fp8 x fp8 -> f32 PSUM),
  - exp with fused accumulation over the class axis (scale folds the
    1/256 quantization scale and the ArcFace scale 64),
  - target logit via a per-row dot with the host-gathered target weight
    row (xw . wtar, 256-wide DVE reduce),
  - the margin + CE epilogue on [128,T] vectors (both tiles batched),
  - one partial-loss scalar out (sum of -logp/B over its samples).

Host: sums the 8 scalars. ~4MB HBM traffic per core => memory-bound.

Samples are packed into "bands" of NG=32 partition rows, one band per
weight group (plus overflow bands), 4 bands per 128-row sample tile.
"""

import sys
import numpy as np
import ml_dtypes

BF16 = ml_dtypes.bfloat16
FP8 = ml_dtypes.float8_e4m3

_TRN_REPO = "/opt/trn_rl_repo"
if _TRN_REPO not in sys.path:
    sys.path.insert(0, _TRN_REPO)

# problem config (hardcoded per spec)
B, E, G, C = 512, 256, 64, 2048
NCORES = 8
GPC = G // NCORES        # weight groups per core
NG = 32                  # sample slots per band
BPT = 128 // NG          # bands per 128-partition sample tile
KE = E // 128            # contraction chunks
NCC = C // 512           # 512-col chunks per group
SCALE = 64.0
MARGIN = 0.5
COS_M = float(np.cos(MARGIN))
SIN_M = float(np.sin(MARGIN))
THETA = float(np.cos(np.pi - MARGIN))
SINMM = float(np.sin(np.pi - MARGIN) * MARGIN)
EPS = 1e-12
WS = 16.0                # fp8 pre-scale (per operand); PSUM = WS^2 * cos
NAUX = 2 * E + 1         # per-tile aux row: xw | wtar | redw
DOUBLE_ROW = False       # fp8 DoubleRow perf mode for the mains

_graph_cache = {}


def _build(nb, double_row=DOUBLE_ROW):
    """Build the per-core Bass graph for nb weight bands (nb % BPT == 0)."""
    from contextlib import ExitStack
    import concourse.bacc as bacc
    import concourse.tile as tile
    from concourse import mybir

    f32 = mybir.dt.float32
    bf16 = mybir.dt.bfloat16
    fp8 = mybir.dt.float8e4
    i32 = mybir.dt.int32
    A = mybir.AluOpType
    AF = mybir.ActivationFunctionType

    T = nb // BPT
    nc = bacc.Bacc(None)

    wt_ext = nc.declare_dram_parameter("wt", [nb, 128, KE, C], fp8, isOutput=False)
    xt_ext = nc.declare_dram_parameter("xt", [128, T, KE, 128], fp8, isOutput=False)
    aux_ext = nc.declare_dram_parameter("aux", [128, T, NAUX], bf16, isOutput=False)
    out_ext = nc.declare_dram_parameter("out", [1, 1], f32, isOutput=True)

    with tile.TileContext(nc) as tc, ExitStack() as ctx:
        wpool = ctx.enter_context(tc.tile_pool(name="w", bufs=nb))
        cpool = ctx.enter_context(tc.tile_pool(name="const", bufs=1))
        vpool = ctx.enter_context(tc.tile_pool(name="vec", bufs=2))
        pmain = ctx.enter_context(tc.tile_pool(name="pmain", bufs=4, space="PSUM"))
        ploss = ctx.enter_context(tc.tile_pool(name="ploss", bufs=1, space="PSUM"))

        # DMA triggers: weight bands alternate sync/scalar HW-DGE queues so
        # descriptor generation is parallel; xt first on scalar (mains need
        # it), aux early for the margin pre-compute.
        w_tiles = []
        for b in range(nb):
            wt = wpool.tile([128, KE, C], fp8, tag="wt", name=f"wt{b}")
            w_tiles.append(wt)
        xt_sb = cpool.tile([128, T, KE, 128], fp8, tag="xt")
        aux_sb = cpool.tile([128, T, NAUX], bf16, tag="aux")

        nc.sync.dma_start(out=w_tiles[0][:], in_=wt_ext[0])
        nc.scalar.dma_start(out=xt_sb[:], in_=xt_ext[:])
        nc.sync.dma_start(out=w_tiles[2][:], in_=wt_ext[2])
        nc.scalar.dma_start(out=w_tiles[1][:], in_=wt_ext[1])
        nc.sync.dma_start(out=w_tiles[4][:], in_=wt_ext[4])
        nc.scalar.dma_start(out=aux_sb[:], in_=aux_ext[:])
        for b in range(3, nb, 2):
            nc.scalar.dma_start(out=w_tiles[b][:], in_=wt_ext[b])
        for b in range(6, nb, 2):
            nc.sync.dma_start(out=w_tiles[b][:], in_=wt_ext[b])

        # preload the natural_log_exp_and_others ACT table set (exp, ln):
        # one resident set => zero mid-kernel table loads. After the scalar
        # queue's DMA triggers so they are not delayed.
        nc.scalar.add_instruction(mybir.InstLoadActFuncSet(
            name="preload-actset-6", act_func_set_id=6, ins=[], outs=[]))

        # margin pre-compute, both tiles batched as [128, T] columns:
        # t = <xn, wn_target>; ft = t>theta ? t*cos_m - sqrt(1-t^2)*sin_m
        #                                  : t - sinmm   (labels always valid)
        tcos = cpool.tile([128, T], f32, tag="tcos")
        for t in range(T):
            tscr = vpool.tile([128, E], f32, tag="tscr")
            nc.vector.tensor_tensor(tscr[:], aux_sb[:, t, 0:E],
                                    aux_sb[:, t, E:2 * E], A.mult)
            nc.vector.reduce_sum(tcos[:, t:t + 1], tscr[:], axis=mybir.AxisListType.X)
        t2 = vpool.tile([128, T], f32, tag="t2")
        nc.vector.tensor_tensor(t2[:], tcos[:], tcos[:], A.mult)
        nc.vector.tensor_scalar(t2[:], t2[:], -1.0, 1.0, op0=A.mult, op1=A.add)
        nc.vector.tensor_scalar_max(t2[:], t2[:], 0.0)
        # sin_t = z*rsqrt(z): Quake seed + 2 Newton iterations on DVE
        yrs = vpool.tile([128, T], f32, tag="yrs")
        yi = yrs.bitcast(i32)
        nc.vector.tensor_scalar(yi[:], t2.bitcast(i32)[:], 1, None, op0=A.arith_shift_right)
        nc.vector.tensor_scalar(yi[:], yi[:], -1, 0x5F3759DF, op0=A.mult, op1=A.add)
        hz = vpool.tile([128, T], f32, tag="hz")
        nc.vector.tensor_scalar_mul(hz[:], t2[:], 0.5)
        y2 = vpool.tile([128, T], f32, tag="y2")
        for _ in range(2):
            nc.vector.tensor_tensor(y2[:], yrs[:], yrs[:], A.mult)
            nc.vector.tensor_tensor(y2[:], y2[:], hz[:], A.mult)
            nc.vector.tensor_scalar(y2[:], y2[:], -1.0, 1.5, op0=A.mult, op1=A.add)
            nc.vector.tensor_tensor(yrs[:], yrs[:], y2[:], A.mult)
        sint = vpool.tile([128, T], f32, tag="sint")
        nc.vector.tensor_tensor(sint[:], t2[:], yrs[:], A.mult)
        ctm = vpool.tile([128, T], f32, tag="ctm")
        nc.vector.tensor_scalar_mul(ctm[:], tcos[:], COS_M)
        sinm = vpool.tile([128, T], f32, tag="sinm")
        nc.vector.tensor_scalar_mul(sinm[:], sint[:], SIN_M)
        nc.vector.tensor_tensor(ctm[:], ctm[:], sinm[:], A.subtract)
        tms = vpool.tile([128, T], f32, tag="tms")
        nc.vector.tensor_scalar_add(tms[:], tcos[:], -SINMM)
        gt = vpool.tile([128, T], i32, tag="gt")
        nc.vector.tensor_scalar(gt[:], tcos[:], THETA, None, op0=A.is_gt)
        ft = vpool.tile([128, T], f32, tag="ft")
        nc.vector.select(ft[:], gt[:], ctm[:], tms[:])
        tf64 = cpool.tile([128, 2 * T], f32, tag="tf64")
        nc.vector.tensor_scalar_mul(tf64[:, 0:T], tcos[:], SCALE)
        nc.vector.tensor_scalar_mul(tf64[:, T:2 * T], ft[:], SCALE)
        eb = cpool.tile([128, 2 * T], f32, tag="eb")
        nc.scalar.activation(eb[:], tf64[:], AF.Exp)
        # per tile t: exp(64t) = eb[:, t], exp(64ft) = eb[:, T+t],
        #             64ft = tf64[:, T+t]

        loss_ps = ploss.tile([1, 1], f32, tag="loss")
        escale = SCALE / (WS * WS)   # exp(escale * psum) = exp(64*cos)

        def emit_tile(t, cc_outer):
            """mains + exp/accum + CE epilogue for sample tile t"""
            cps = [pmain.tile([128, 512], f32, tag="cos", name=f"cos{t}_{cc}")
                   for cc in range(NCC)]
            if double_row:
                order = ([(cc, j) for cc in range(NCC) for j in range(BPT)]
                         if cc_outer else
                         [(cc, j) for j in range(BPT) for cc in range(NCC)])
                for cc, j in order:
                    nc.tensor.matmul(
                        cps[cc][NG * j:NG * (j + 1), :],
                        xt_sb[:, t, 0:KE, NG * j: NG * (j + 1)],
                        w_tiles[BPT * t + j][:, 0:KE, 512 * cc: 512 * cc + 512],
                        start=True, stop=True,
                        perf_mode=mybir.MatmulPerfMode.DoubleRow,
                        tile_position=(0, NG * j),
                    )
            else:
                order = ([(cc, j, k) for cc in range(NCC) for j in range(BPT) for k in range(KE)]
                         if cc_outer else
                         [(cc, j, k) for j in range(BPT) for k in range(KE) for cc in range(NCC)])
                for cc, j, k in order:
                    nc.tensor.matmul(
                        cps[cc][NG * j:NG * (j + 1), :],
                        xt_sb[:, t, k, NG * j: NG * (j + 1)],
                        w_tiles[BPT * t + j][:, k, 512 * cc: 512 * cc + 512],
                        start=(k == 0), stop=(k == KE - 1),
                        tile_position=(0, NG * j),
                    )
            # exp with fused class-axis accumulation, one per PSUM chunk
            ses = cpool.tile([128, NCC], f32, tag=f"ses{t}")
            for cc in range(NCC):
                escr = vpool.tile([128, 512], bf16, tag="escr")
                nc.scalar.activation(escr[:], cps[cc][:], AF.Exp, scale=escale,
                                     accum_out=ses[:, cc:cc + 1])
            sumexp = vpool.tile([128, 1], f32, tag="sumexp")
            nc.vector.reduce_sum(sumexp[:], ses[:], axis=mybir.AxisListType.X)
            # se2 = sumexp - exp(64 t) + exp(64 ft);  lb = ln(se2) - 64 ft
            se2 = vpool.tile([128, 1], f32, tag="se2")
            nc.vector.tensor_tensor(se2[:], sumexp[:], eb[:, t:t + 1], A.subtract)
            nc.vector.tensor_tensor(se2[:], se2[:], eb[:, T + t:T + t + 1], A.add)
            lse = vpool.tile([128, 1], f32, tag="lse")
            nc.scalar.activation(lse[:], se2[:], AF.Ln)
            lb = cpool.tile([128, 1], bf16, tag=f"lb{t}")
            nc.vector.tensor_tensor(lb[:], lse[:], tf64[:, T + t:T + t + 1], A.subtract)
            nc.tensor.matmul(
                loss_ps[:], aux_sb[:, t, 2 * E:NAUX], lb[:],
                start=(t == 0), stop=(t == T - 1),
            )

        emit_tile(0, cc_outer=False)
        for t in range(1, T):
            emit_tile(t, cc_outer=True)

        loss_sb = cpool.tile([1, 1], f32, tag="losssb")
        nc.vector.tensor_copy(loss_sb[:], loss_ps[:])
        nc.sync.dma_start(out=out_ext[:], in_=loss_sb[:])

    nc.compile()
    return nc


def _pack(logits, labels, weight):
    """Route samples to the core owning their group; build per-core inputs."""
    logits = np.asarray(logits, dtype=np.float32)
    labels = np.asarray(labels).astype(np.int64)
    weight = np.asarray(weight, dtype=np.float32)

    group = (labels // C).astype(np.int64)
    local = (labels % C).astype(np.int64)
    core = group // GPC
    gl = group % GPC

    # host-side l2 normalization + fp8 quantization (x16 keeps the values
    # in fp8e4's normal range; cos is invariant to the row scaling)
    xn = logits / np.maximum(
        np.sqrt(np.sum(logits * logits, axis=1, keepdims=True)), EPS)
    wn2 = np.sqrt(np.einsum("gce,gce->gc", weight, weight))[:, :, None]
    wn = weight / np.maximum(wn2, EPS)
    wq = (WS * wn).astype(FP8)                    # (G, C, E) fp8 table
    xq = (WS * xn).astype(FP8)                    # (B, E)
    xw_all = (xq.astype(np.float32) / WS).astype(BF16)
    wtar_all = (wq[group, local].astype(np.float32) / WS).astype(BF16)

    # band assignment: per (core, local-group), ceil(count/NG) bands
    percg = [[np.nonzero((core == c) & (gl == g))[0] for g in range(GPC)]
             for c in range(NCORES)]
    nbands = [sum(max(1, -(-len(idx) // NG)) for idx in percg[c])
              for c in range(NCORES)]
    nb = max(nbands)
    nb = -(-nb // BPT) * BPT  # round up to full sample tiles
    T = nb // BPT

    in_maps = []
    for c in range(NCORES):
        # band -> (group, sample indices)
        bands = []
        for g in range(GPC):
            idx = percg[c][g]
            nslice = max(1, -(-len(idx) // NG))
            for s in range(nslice):
                bands.append((g, idx[s * NG:(s + 1) * NG]))
        while len(bands) < nb:
            bands.append((0, np.empty(0, dtype=np.int64)))

        wt = np.empty((nb, 128, KE, C), dtype=FP8)
        xqp = np.zeros((T, 128, E), dtype=FP8)
        aux = np.zeros((128, T, NAUX), dtype=BF16)
        for b, (g, idx) in enumerate(bands):
            wg = wq[c * GPC + g]                     # (C, E) fp8
            for k in range(KE):
                wt[b, :, k, :] = wg[:, k * 128:(k + 1) * 128].T
            t, j = b // BPT, b % BPT
            sl = slice(NG * j, NG * j + len(idx))
            xqp[t, sl, :] = xq[idx]
            aux[sl, t, 0:E] = xw_all[idx]
            aux[sl, t, E:2 * E] = wtar_all[idx]
            aux[sl, t, 2 * E] = BF16(1.0 / B)
        # xt[p, t, k, r] = xq[t][r, k*128+p]
        xt = np.ascontiguousarray(
            np.transpose(xqp.reshape(T, 128, KE, 128), (3, 0, 2, 1)))
        in_maps.append({"wt": wt, "xt": xt, "aux": aux})
    return in_maps, nb


def _run(logits, labels, weight, trace=False, **kw):
    from concourse.bass_utils import run_bass_kernel_spmd

    in_maps, nb = _pack(logits, labels, weight)
    nc = _graph_cache.get(nb)
    if nc is None:
        nc = _build(nb)
        _graph_cache[nb] = nc
    res = run_bass_kernel_spmd(nc, in_maps, core_ids=list(range(NCORES)),
                               trace=trace, **kw)
    total = sum(float(res.results[i]["out"][0, 0]) for i in range(NCORES))
    return np.asarray(total, dtype=np.float32), res


def kernel(logits, labels, weight):
    loss, _ = _run(logits, labels, weight)
    return loss
